# revision 9
# baseline (speedup 1.0000x reference)
"""nn_Detection_CrossEntropy Trainium2 kernel (8 NeuronCores, pure data parallel).

Each core processes one sample b of output[B=8, N=25200, 85] in row-windows
of [25,50,50,50,25] (128 partitions x W rows).

v2: fp16 datapath. The IoU mask is computed in g-major [P, G, W] layout so
every big DVE tensor_tensor op has a packed (stride-1) last dim and hits the
2x_1p fp16 fast path; relu*3 uses tensor_scalar (4x_2p). GT-side operands are
materialized once as [P, G*50] fp16 tiles. The exp-sum reduce is a halving
add-tree (2x fp16) instead of tensor_reduce (1x). Matmuls run in fp16
(1 cycle/row vs 4 for fp32): psum_T[32,82] += mask_r^T @ [obj*logits | LSE | 1].
Work is balanced DVE vs GPSIMD (premult split, y-mins + GP add on GPSIMD).
loss_b = (sum(T[:,80]) - sum_g T[g, cls_g]) / sum(T[:,81]) on host.
"""
import numpy as np

"""Workaround: this container's walrus rejects >2 sync waits on the
TileContext tail Drain (setupSyncWait<CTRL_NO_STRUCT>: "Too many sync
wait commands"). Split the tail-drain waits across multiple drains."""
import concourse.mybir as mybir
from concourse import tile
from concourse.vector_clock import ScopedClock

MAXW = 1

def _drain_and_barrier(self, tick_clock, wait_clock):
    nc = self.nc
    drain_inst = nc.sync.drain()
    wait_clock.add_sem_waits(drain_inst.ins, ScopedClock({None: tick_clock.global_clock}))
    si = drain_inst.ins.sync_info
    if si is not None and si.on_wait is not None and len(si.on_wait) > MAXW:
        waits = list(si.on_wait)
        si.on_wait = waits[:MAXW]
        for i in range(MAXW, len(waits), MAXW):
            extra = nc.sync.drain()
            esi = extra.ins.sync_info
            if esi is None:
                extra.ins.sync_info = mybir.SyncInfo(on_wait=waits[i:i+MAXW], on_update=[])
            else:
                esi.on_wait = waits[i:i+MAXW]
    nc.all_engine_barrier()
    assert self.sems is not None
    popped = nc._tile_sem_poison_stack.pop()
    assert popped is self._sem_poison
    nc.clear_and_free_semaphores(list(self.sems.allocated().values()))
    nc.all_engine_barrier()

tile.TileContext._drain_and_barrier = _drain_and_barrier


# General fix: this walrus accepts at most ONE sync wait per instruction.
# Split extra waits onto preceding Drain carriers at BIR-JSON level.
import orjson
import concourse.bass as _bass

_orig_to_json_bytes = _bass.Bass.to_json_bytes

def _to_json_bytes_split(self) -> bytes:
    j = orjson.loads(_orig_to_json_bytes(self))
    for f in j.get("functions", []):
        for bb in f.get("blocks", []):
            out = []
            changed = False
            for i in bb.get("instructions", []):
                si = i.get("sync_info")
                ow = (si or {}).get("on_wait") or []
                if len(ow) > 1:
                    changed = True
                    for k, w in enumerate(ow[:-1]):
                        out.append({
                            "name": f'{i["name"]}-w{k}',
                            "opcode": "Drain",
                            "engine": i["engine"],
                            "ins": [],
                            "outs": [],
                            "debug": i.get("debug", 0),
                            "sync_info": {"on_update": [], "on_wait": [w]},
                        })
                    si["on_wait"] = [ow[-1]]
                out.append(i)
            if changed:
                bb["instructions"] = out
    return orjson.dumps(j)

_bass.Bass.to_json_bytes = _to_json_bytes_split


# kernel builder:

import concourse.bass as bass

F32 = mybir.dt.float32
F16 = mybir.dt.float16
ALU = mybir.AluOpType
ACTF = mybir.ActivationFunctionType

N, G, C = 25200, 32, 80
NPAD = 25600
P = 128
R = NPAD // P            # 200 rows per partition
ROW = 85
SCALE = 640.0
WINDOWS = [36, 64, 64, 36]
WMAX = max(WINDOWS)
FEATW = C + 2            # 82: [obj*logits | lse | 1]


def build_kernel(outer=1, gps_premult_frac=0.95, y_on_gps=False, gp_on_gps=True):
    nc = bass.Bass()
    data = nc.declare_dram_parameter("data", [P, R * ROW], F32, isOutput=False)
    lb = nc.declare_dram_parameter("lb", [G, 5], F32, isOutput=False)
    res = nc.declare_dram_parameter("res", [G, FEATW], F32, isOutput=True)
    gt_bounce = nc.dram_tensor("gt_bounce", [G * 5], F32)

    with tile.TileContext(nc) as tc:
        with (
            tc.tile_pool(name="const", bufs=1) as constp,
            tc.tile_pool(name="main", bufs=2) as mainp,
            tc.tile_pool(name="feat", bufs=2) as featp,
            tc.tile_pool(name="cols", bufs=2) as colsp,
            tc.tile_pool(name="pair", bufs=2) as pairp,
            tc.tile_pool(name="psum", bufs=1, space="PSUM") as psump,
        ):
          for _o in range(outer):
            # ---------------- GT prep (once per sample) ----------------
            lbt = constp.tile([G, 5], F32, name="lbt")
            nc.sync.dma_start(lbt[:], lb[:, :])
            gx, gy = lbt[:, 1:2], lbt[:, 2:3]
            gw, gh = lbt[:, 3:4], lbt[:, 4:5]
            pack = constp.tile([G, 5], F32, name="pack")  # gx1n,gx2,gy1n,gy2,ga
            raw = constp.tile([G, 4], F32, name="raw")
            nc.vector.scalar_tensor_tensor(raw[:, 0:1], gw, -0.5, gx, ALU.mult, ALU.add)
            nc.vector.scalar_tensor_tensor(raw[:, 1:2], gw, 0.5, gx, ALU.mult, ALU.add)
            nc.vector.scalar_tensor_tensor(raw[:, 2:3], gh, -0.5, gy, ALU.mult, ALU.add)
            nc.vector.scalar_tensor_tensor(raw[:, 3:4], gh, 0.5, gy, ALU.mult, ALU.add)
            clp = constp.tile([G, 4], F32, name="clp")
            nc.vector.tensor_scalar(clp[:], raw[:], 0.0, 1.0, ALU.max, ALU.min)
            nc.vector.tensor_scalar_mul(pack[:, 0:4], clp[:], SCALE)
            wt = constp.tile([G, 2], F32, name="wt")
            nc.vector.tensor_sub(wt[:, 0:1], pack[:, 1:2], pack[:, 0:1])
            nc.vector.tensor_sub(wt[:, 1:2], pack[:, 3:4], pack[:, 2:3])
            nc.vector.tensor_mul(pack[:, 4:5], wt[:, 0:1], wt[:, 1:2])
            # negate gx1, gy1 in place (cols 0, 2) for the min/add-form chain
            nc.vector.tensor_scalar_mul(pack[:, 0:1], pack[:, 0:1], -1.0)
            nc.vector.tensor_scalar_mul(pack[:, 2:3], pack[:, 2:3], -1.0)
            nc.sync.dma_start(gt_bounce[:].rearrange("(q g) -> g q", g=G), pack[:])
            gt_bc = constp.tile([P, 5 * G], F32, name="gt_bc")
            nc.sync.dma_start(gt_bc[:], gt_bounce[:][None, :].partition_broadcast(P))
            gt16 = constp.tile([P, 5 * G], F16, name="gt16")
            nc.vector.tensor_copy(gt16[:], gt_bc[:])
            # materialize each gt value along a packed W axis: [P, G, WMAX] f16
            # (broadcast copies run on the otherwise-idle Act engine)
            gmat = constp.tile([P, 5 * G * WMAX], F16, name="gmat")
            gm3 = gmat[:].rearrange("p (q g w) -> p q g w", q=5, g=G)
            for q in range(5):
                nc.scalar.activation(
                    gm3[:, q],
                    gt16[:, q * G:(q + 1) * G][:, :, None].broadcast_to([P, G, WMAX]),
                    ACTF.Copy,
                )
            gx1n_m, gx2_m = gm3[:, 0], gm3[:, 1]
            gy1n_m, gy2_m = gm3[:, 2], gm3[:, 3]
            ga_m = gm3[:, 4]

            psum_T = psump.tile([G, FEATW], F32, name="psum_T")

            r_base = 0
            for w, W in enumerate(WINDOWS):
                mt = mainp.tile([P, W * ROW], F32, tag="mt", name="mt")
                nc.sync.dma_start(mt[:], data[:, r_base * ROW:(r_base + W) * ROW])
                m3 = mt[:].rearrange("p (r c) -> p r c", c=ROW)
                x_c, y_c = m3[:, :, 0], m3[:, :, 1]
                w_c, h_c = m3[:, :, 2], m3[:, :, 3]
                obj_b = m3[:, :, 4][:, :, None].broadcast_to([P, W, C])

                # packed f16 per-pred columns: px1n, px2, py1n, py2, pa
                cols = colsp.tile([P, 5 * W], F16, tag="cols", name="cols")
                c3 = cols[:].rearrange("p (q r) -> p q r", q=5)
                px1n, px2 = c3[:, 0, :], c3[:, 1, :]
                py1n, py2 = c3[:, 2, :], c3[:, 3, :]
                pa = c3[:, 4, :]
                nc.vector.scalar_tensor_tensor(px1n, w_c, 0.5, x_c, ALU.mult, ALU.subtract)
                nc.vector.scalar_tensor_tensor(px2, w_c, 0.5, x_c, ALU.mult, ALU.add)
                nc.vector.scalar_tensor_tensor(py1n, h_c, 0.5, y_c, ALU.mult, ALU.subtract)
                nc.vector.scalar_tensor_tensor(py2, h_c, 0.5, y_c, ALU.mult, ALU.add)
                nc.vector.tensor_mul(pa, w_c, h_c)

                # ---- feat = [obj*logits | lse | 1] in f16, w-major ----
                feat = featp.tile([P, W * FEATW], F16, tag="feat", name="feat")
                f3 = feat[:].rearrange("p (r c) -> p r c", c=FEATW)
                wsplit = int(round(W * gps_premult_frac))
                if wsplit > 0:
                    nc.gpsimd.tensor_tensor(
                        f3[:, 0:wsplit, 0:C], m3[:, 0:wsplit, 5:ROW],
                        obj_b[:, 0:wsplit], ALU.mult)
                if wsplit < W:
                    nc.vector.tensor_tensor(
                        f3[:, wsplit:W, 0:C], m3[:, wsplit:W, 5:ROW],
                        obj_b[:, wsplit:W], ALU.mult)
                nc.gpsimd.memset(f3[:, :, C + 1], 1.0)

                # exp (Act) then halving-tree row sums (DVE, f16 2x)
                et = featp.tile([P, W * C], F16, tag="et", name="et")
                nc.scalar.activation(et[:], f3[:, :, 0:C], ACTF.Exp)
                e3 = et[:].rearrange("p (r c) -> p r c", c=C)
                t40 = colsp.tile([P, W * 40], F16, tag="t40", name="t40")
                h3 = t40[:].rearrange("p (r c) -> p r c", c=40)
                nc.vector.tensor_tensor(h3[:, :, 0:40], e3[:, :, 0:40], e3[:, :, 40:80], ALU.add)
                nc.vector.tensor_tensor(h3[:, :, 0:20], h3[:, :, 0:20], h3[:, :, 20:40], ALU.add)
                nc.vector.tensor_tensor(h3[:, :, 0:10], h3[:, :, 0:10], h3[:, :, 10:20], ALU.add)
                nc.vector.tensor_tensor(h3[:, :, 0:5], h3[:, :, 0:5], h3[:, :, 5:10], ALU.add)
                nc.vector.tensor_tensor(h3[:, :, 0:2], h3[:, :, 0:2], h3[:, :, 2:4], ALU.add)
                sums = colsp.tile([P, W], F32, tag="sums", name="sums")
                nc.vector.tensor_tensor(sums[:][:, :, None], h3[:, :, 0:1], h3[:, :, 1:2], ALU.add)
                nc.vector.tensor_tensor(sums[:][:, :, None], sums[:][:, :, None], h3[:, :, 4:5], ALU.add)
                lsew = colsp.tile([P, W], F32, tag="lsew", name="lsew")
                nc.scalar.activation(lsew[:], sums[:], ACTF.Ln)
                nc.scalar.activation(f3[:, :, C], lsew[:], ACTF.Copy)

                # ---- IoU mask, g-major [P, G, W] f16 ----
                def pbr(col):  # per-pred col broadcast over G (last dim packed)
                    return col[:, None, :].broadcast_to([P, G, W])
                gsl = lambda t: t[:, :, 0:W]
                sh = lambda t: t[:].rearrange("p (g w) -> p g w", g=G)

                Ax = pairp.tile([P, G * W], F16, tag="Ax", name="Ax")
                Bx = pairp.tile([P, G * W], F16, tag="Bx", name="Bx")
                Cy = pairp.tile([P, G * W], F16, tag="Cy", name="Cy")
                Dy = pairp.tile([P, G * W], F16, tag="Dy", name="Dy")
                MK = pairp.tile([P, G * W], F16, tag="MK", name="MK")

                nc.vector.tensor_tensor(sh(Bx), pbr(px2), gsl(gx2_m), ALU.min)
                nc.vector.tensor_tensor(sh(Ax), pbr(px1n), gsl(gx1n_m), ALU.min)
                # GPSIMD ucode on V3 lowers only add/subtract/mult TensorTensor;
                # min/is_ge must stay on DVE (walrus engine check rejects them).
                nc.vector.tensor_tensor(sh(Dy), pbr(py2), gsl(gy2_m), ALU.min)
                nc.vector.tensor_tensor(sh(Cy), pbr(py1n), gsl(gy1n_m), ALU.min)
                nc.vector.tensor_tensor(Bx[:], Bx[:], Ax[:], ALU.add)       # wx
                nc.vector.tensor_scalar(Bx[:], Bx[:], 0.0, 3.0, ALU.max, ALU.mult)  # 3*relu(wx)
                nc.vector.tensor_tensor(Dy[:], Dy[:], Cy[:], ALU.add)       # wy
                nc.vector.tensor_tensor(Bx[:], Bx[:], Dy[:], ALU.mult)      # V
                gp_eng = nc.gpsimd if gp_on_gps else nc.vector
                gp_eng.tensor_tensor(sh(Cy), pbr(pa), gsl(ga_m), ALU.add)   # GP
                nc.vector.tensor_tensor(MK[:], Bx[:], Cy[:], ALU.is_ge)     # mask

                mk3 = MK[:].rearrange("p (g w) -> p g w", g=G)
                for rr in range(W):
                    r = r_base + rr
                    nc.tensor.matmul(
                        psum_T[:],
                        mk3[:, :, rr],
                        f3[:, rr, :],
                        start=(r == 0),
                        stop=(r == R - 1),
                    )
                r_base += W

            out_t = constp.tile([G, FEATW], F32, name="out_t")
            nc.scalar.activation(out_t[:], psum_T[:], ACTF.Copy)
            nc.sync.dma_start(res[:, :], out_t[:])
    return nc


def host_finish(res_list, label_batch):
    B = len(res_list)
    out = np.empty((1, B), np.float32)
    for b in range(B):
        T = res_list[b]
        cls = np.asarray(label_batch)[b, :, 0].astype(np.int32)
        S_T = T[np.arange(G), cls].sum()
        S_L = T[:, C].sum()
        S_0 = T[:, C + 1].sum()
        out[0, b] = (S_L - S_T) / S_0
    return out


def prep_inputs(output, label_batch):
    B = output.shape[0]
    pad = np.zeros((B, NPAD - N, ROW), output.dtype)
    data = np.concatenate([np.asarray(output), pad], axis=1)
    data = data.reshape(B, P, R * ROW)
    return [{"data": data[b], "lb": np.asarray(label_batch[b])} for b in range(B)]


_CACHE = {}


def kernel(output, label_batch, prob_threshold):
    """Full inputs -> [1, B] loss. prob_threshold == 0 for this problem
    (keep = obj >= 0 is always true; padded rows are masked geometrically)."""
    from concourse.bass_utils import run_bass_kernel_spmd

    output = np.asarray(output)
    label_batch = np.asarray(label_batch)
    B = output.shape[0]
    if "nc" not in _CACHE:
        _CACHE["nc"] = build_kernel()
    nc = _CACHE["nc"]
    in_maps = prep_inputs(output, label_batch)
    r = run_bass_kernel_spmd(nc, in_maps, list(range(B)))
    res_list = [r.results[b]["res"] for b in range(B)]
    return host_finish(res_list, label_batch).astype(output.dtype)
